# revision 2
# baseline (speedup 1.0000x reference)
import numpy as np
import jax
import jax.numpy as jnp

# nn_GaussianHeatmapLoss — data-parallel over batch across 8 NeuronCores.
# Full shapes (hardcoded per contract): source (32,14,256,256) f32,
# target (32,14,2) f32. Output: (scalar mse loss, heatmap (32,14,256,256)).
STD = 1.5
LANDMARK_DOWNSCALE = 1.0
EPS = 1e-6
B, K, H, W = 32, 14, 256, 256
M = 8  # cores; shard B -> M x (B//M)


def _shard_fn(source, target):
    # source: (b,K,H,W); target: (b,K,2)
    # Separable form: exp(-(dr^2+dc^2)/s) == exp(-dr^2/s) * exp(-dc^2/s),
    # and max over the grid factorizes as max(gy)*max(gx), so the big
    # (N,H,W) distance tensor is never materialized.
    coords = (target / LANDMARK_DOWNSCALE).reshape(-1, 2)
    rows = jnp.arange(H, dtype=coords.dtype)
    cols = jnp.arange(W, dtype=coords.dtype)
    cx = coords[:, 0][:, None]
    cy = coords[:, 1][:, None]
    inv = 1.0 / (2.0 * STD * STD)
    gy = jnp.exp(-((rows[None, :] - cy) ** 2) * inv)  # (N,H)
    gx = jnp.exp(-((cols[None, :] - cx) ** 2) * inv)  # (N,W)
    mx = jnp.max(gy, axis=1, keepdims=True) * jnp.max(gx, axis=1, keepdims=True)
    scale = jnp.where(mx > EPS, 1.0 / mx, 1.0)
    valid = ~(jnp.isnan(coords[:, 0]) | jnp.isnan(coords[:, 1]))
    gy = jnp.where(valid[:, None], gy * scale, 0.0)
    hm = (gy[:, :, None] * gx[:, None, :]).reshape(source.shape)
    sse = jnp.sum((source - hm) ** 2)  # partial sum; reduced on host
    return sse, hm


_pmapped = jax.pmap(_shard_fn, devices=jax.devices()[:M])


def kernel(source, target):
    src = np.ascontiguousarray(np.asarray(source, dtype=np.float32)).reshape(
        M, B // M, K, H, W
    )
    tgt = np.ascontiguousarray(np.asarray(target, dtype=np.float32)).reshape(
        M, B // M, K, 2
    )
    sse, hm = _pmapped(src, tgt)
    loss = np.asarray(
        np.sum(np.asarray(sse, dtype=np.float64)) / (B * K * H * W), dtype=np.float32
    )
    heatmap = np.asarray(hm).reshape(B, K, H, W)
    return loss, heatmap


# revision 3
# speedup vs baseline: 2.1940x; 2.1940x over previous
import numpy as np
import jax
import jax.numpy as jnp

# nn_GaussianHeatmapLoss — data-parallel over batch across 8 NeuronCores.
# Full shapes (hardcoded per contract): source (32,14,256,256) f32,
# target (32,14,2) f32. Output: (scalar mse loss, heatmap (32,14,256,256)).
STD = 1.5
LANDMARK_DOWNSCALE = 1.0
EPS = 1e-6
B, K, H, W = 32, 14, 256, 256
M = 8  # cores; shard B -> M x (B//M)


def _shard_fn(source, target):
    # source: (b,K,H,W); target: (b,K,2)
    coords = (target / LANDMARK_DOWNSCALE).reshape(-1, 2)
    rows = jnp.arange(H, dtype=coords.dtype)
    cols = jnp.arange(W, dtype=coords.dtype)
    cx = coords[:, 0][:, None, None]
    cy = coords[:, 1][:, None, None]
    d2 = (rows[None, :, None] - cy) ** 2 + (cols[None, None, :] - cx) ** 2
    hm = jnp.exp(-d2 / (2.0 * STD * STD))
    mx = jnp.max(hm, axis=(1, 2), keepdims=True)
    hm = jnp.where(mx > EPS, hm / mx, hm)
    valid = ~(jnp.isnan(coords[:, 0]) | jnp.isnan(coords[:, 1]))
    hm = jnp.where(valid[:, None, None], hm, 0.0)
    hm = hm.reshape(source.shape)
    sse = jnp.sum((source - hm) ** 2)  # partial sum; reduced on host
    return sse, hm


_pmapped = jax.pmap(_shard_fn, devices=jax.devices()[:M])


def kernel(source, target):
    src = np.ascontiguousarray(np.asarray(source, dtype=np.float32)).reshape(
        M, B // M, K, H, W
    )
    tgt = np.ascontiguousarray(np.asarray(target, dtype=np.float32)).reshape(
        M, B // M, K, 2
    )
    sse, hm = _pmapped(src, tgt)
    loss = np.asarray(
        np.sum(np.asarray(sse, dtype=np.float64)) / (B * K * H * W), dtype=np.float32
    )
    heatmap = np.asarray(hm).reshape(B, K, H, W)
    return loss, heatmap
